# revision 23
# baseline (speedup 1.0000x reference)
"""Trainium2 Bass kernel for nn_DecoderCrossAttention.

Reference computation (per voxel v, batch b):
    q = Wq x_v + bq                        (x = decoder_features, [C])
    k_j = Wk y_jv + bk, v_j = Wv y_jv + bv (y = skip features, COND=4 frames)
    s_j[h] = <q_h, k_jh> / sqrt(DH)        (NH=8 heads of DH=16)
    attn = softmax_j(s)                    (over the 4 conditioning frames)
    o = Wo (sum_j attn_j * v_j) + bo + x_v
    out = GroupNorm8(o) * gamma + beta     (stats over (C/G, H, W, D) per batch)

Strategy (8 NeuronCores, data-parallel over H):
  * Each core gets H-slice of 4 planes: 2*4*32*32 = 8192 voxels.
  * Feature-major layout [C=128 partitions, voxels in free dim], 512-voxel tiles.
  * All projections are PE matmuls in float32r (full rate at N=512).
  * Per-head score reduction (sum over the 16 channels of a head) and the
    softmax broadcast (8 head rows -> 128 channels) are PE matmuls against
    0/1 masks built in-kernel with iota+compare.
  * Softmax over only 4 logits, inputs are bounded => no max subtraction.
  * E~ = exp(s)*recip(Z) is computed on [32, NT] tiles (4 conds x 8 heads
    stacked on partitions, Z replicated per block by the Z-matmul mask).
  * attn*V products feed 4 accumulating output-projection matmuls (no adds).
  * Residual + bias + per-channel GN sums fused in one scalar_tensor_tensor.
  * GroupNorm is global: per-channel sum/sumsq AllReduce (2KB) across cores,
    then a fused scale+bias pass over the accumulated output.

The walrus build here accepts only ONE sync wait per instruction; Tile
attaches many.  split_waits() hoists extras onto standalone EventSemaphore
instructions post-scheduling.
"""

import sys

if "/opt/trn_rl_repo" not in sys.path:
    sys.path.insert(0, "/opt/trn_rl_repo")

import numpy as np

B, COND, C, H, W, D = 2, 4, 128, 32, 32, 32
NH, DH, G = 8, 16, 8
EPS = 1e-5
NCORES = 8
HS = H // NCORES          # 4 H-planes per core
NVOX = HS * W * D         # 4096 voxels per batch per core
NT = 512                  # voxels per tile
NTILES = NVOX // NT       # 8 tiles per batch
N_GROUP = (C // G) * H * W * D   # elements per (batch, group) for GN stats

_CACHE = {}


def _split_waits(nc):
    """Hoist extra sync waits onto standalone EventSemaphore instructions."""
    from concourse import mybir
    import bass_rust

    n_split = 0
    for func in nc.m.functions:
        for blk in func.blocks:
            new_list = []
            changed = False
            for inst in blk.instructions:
                si = inst.sync_info
                waits = list(si.on_wait) if si is not None else []
                if len(waits) > 1:
                    changed = True
                    for w in waits[:-1]:
                        ev = mybir.InstEventSemaphore(
                            name=f"wsplit-{nc.next_id()}", ins=[], outs=[]
                        )
                        ev.engine = inst.engine
                        ev.sync_info = bass_rust.SyncInfo(on_wait=[w], on_update=[])
                        new_list.append(ev)
                        n_split += 1
                    inst.sync_info = bass_rust.SyncInfo(
                        on_wait=[waits[-1]], on_update=list(si.on_update)
                    )
                new_list.append(inst)
            if changed:
                blk.instructions = new_list
    return n_split


def _build(n_reps=1):
    import concourse.bass as bass
    import concourse.tile as tile
    from concourse import mybir
    from contextlib import ExitStack

    dt = mybir.dt
    f32 = dt.float32
    f32r = dt.float32r
    i32 = dt.int32
    Alu = mybir.AluOpType
    Act = mybir.ActivationFunctionType
    ts = bass.ts

    nc = bass.Bass("TRN2", target_bir_lowering=False, debug=False,
                   num_devices=NCORES)
    x_io = nc.dram_tensor("x", [B, C, NVOX], f32r, kind="ExternalInput").ap()
    y_io = nc.dram_tensor("y", [B, COND, C, NVOX], f32r, kind="ExternalInput").ap()
    w_io = {}
    for name in ("wq", "wk", "wv", "wo"):
        w_io[name] = nc.dram_tensor(name, [C, C], f32r, kind="ExternalInput").ap()
    v_io = {}
    for name in ("bq", "bk", "bv", "bo", "gamma", "beta"):
        v_io[name] = nc.dram_tensor(name, [C, 1], f32, kind="ExternalInput").ap()
    out_io = nc.dram_tensor("out", [B, C, NVOX], f32, kind="ExternalOutput").ap()

    def mm(out, lhsT, rhs, start=True, stop=True):
        nc.tensor.matmul(out, lhsT=lhsT, rhs=rhs, start=start, stop=stop)

    with tile.TileContext(nc) as tc, ExitStack() as ctx:
        # ---------------- constants / weights / masks -------------------
        const = ctx.enter_context(tc.tile_pool(name="const", bufs=1))
        dram = ctx.enter_context(tc.tile_pool(name="dram", bufs=1, space="DRAM"))

        vecs = {}
        for name, io in v_io.items():
            t = const.tile([C, 1], f32, tag=f"vec_{name}")
            nc.sync.dma_start(t[:], io[:])
            vecs[name] = t

        # --- masks via iota + compare (int32), cast to f32
        with tc.tile_pool(name="setup", bufs=1) as setup:
            def icast(dst_ap, src_ap):
                nc.vector.tensor_copy(dst_ap, src_ap)

            # partition-index and free-index helpers
            p128 = setup.tile([C, C], i32, tag="p128")
            nc.gpsimd.iota(p128[:], pattern=[[0, C]], base=0, channel_multiplier=1)
            f128 = setup.tile([C, C], i32, tag="f128")
            nc.gpsimd.iota(f128[:], pattern=[[1, C]], base=0, channel_multiplier=0)
            hc128 = setup.tile([C, C], i32, tag="hc128")
            nc.vector.tensor_scalar(hc128[:], p128[:], 4, None,
                                    Alu.arith_shift_right)
            tmpi = setup.tile([C, C], i32, tag="tmpi")

            # identity [128,128] (for PE transpose of the weights)
            ident = const.tile([C, C], f32r, tag="ident")
            nc.vector.tensor_tensor(tmpi[:], f128[:], p128[:], Alu.is_equal)
            icast(ident[:], tmpi[:])

            # mask32 [128, 4*32]: col 32j+m ; 1 iff (m - 8j) == c//16
            jm = setup.tile([C, C], i32, tag="jm")
            nc.gpsimd.iota(jm[:].rearrange("p (j m) -> p j m", j=4),
                           pattern=[[-8, 4], [1, 32]], base=0,
                           channel_multiplier=0)
            mask32 = const.tile([C, C], f32r, tag="mask32")
            nc.vector.tensor_tensor(tmpi[:], jm[:], hc128[:], Alu.is_equal)
            icast(mask32[:], tmpi[:])

            # lhsT32 [32,32]: 1 iff p%8 == m%8  (Z replication matmul)
            p32 = setup.tile([32, 32], i32, tag="p32")
            nc.gpsimd.iota(p32[:], pattern=[[0, 32]], base=0, channel_multiplier=1)
            pm32 = setup.tile([32, 32], i32, tag="pm32")
            nc.vector.tensor_scalar(pm32[:], p32[:], 3, 3,
                                    Alu.arith_shift_right, Alu.arith_shift_left)
            t32 = setup.tile([32, 32], i32, tag="t32")
            nc.vector.tensor_tensor(t32[:], p32[:], pm32[:], Alu.subtract)
            fm32 = setup.tile([32, 32], i32, tag="fm32")
            nc.gpsimd.iota(fm32[:].rearrange("p (j m) -> p j m", j=4),
                           pattern=[[0, 4], [1, 8]], base=0, channel_multiplier=0)
            e32 = setup.tile([32, 32], i32, tag="e32")
            nc.vector.tensor_tensor(e32[:], fm32[:], t32[:], Alu.is_equal)
            lhsT32 = const.tile([32, 32], f32r, tag="lhsT32")
            icast(lhsT32[:], e32[:])

            # maskb [32, 4*128]: col 128j+c ; 1 iff (p - 8j) == c//16
            pj = setup.tile([32, 4 * C], i32, tag="pj")
            nc.gpsimd.iota(pj[:].rearrange("p (j c) -> p j c", j=4),
                           pattern=[[-8, 4], [0, C]], base=0,
                           channel_multiplier=1)
            fc = setup.tile([32, 4 * C], i32, tag="fc")
            nc.gpsimd.iota(fc[:].rearrange("p (j c) -> p j c", j=4),
                           pattern=[[0, 4], [1, C]], base=0, channel_multiplier=0)
            nc.vector.tensor_scalar(fc[:], fc[:], 4, None, Alu.arith_shift_right)
            eb = setup.tile([32, 4 * C], i32, tag="eb")
            nc.vector.tensor_tensor(eb[:], pj[:], fc[:], Alu.is_equal)
            maskb = const.tile([32, 4 * C], f32r, tag="maskb")
            icast(maskb[:], eb[:])

            # gmask [128, 8]: 1 iff c//16 == g   (GN group reduction)
            g8 = setup.tile([C, 8], i32, tag="g8")
            nc.gpsimd.iota(g8[:], pattern=[[1, 8]], base=0, channel_multiplier=0)
            e8 = setup.tile([C, 8], i32, tag="e8")
            nc.vector.tensor_tensor(e8[:], g8[:], hc128[:, 0:8], Alu.is_equal)
            gmask = const.tile([C, 8], f32, tag="gmask")
            icast(gmask[:], e8[:])

            # gm2 [8, 128]: 1 iff p == c//16    (GN group -> channel bcast)
            p8 = setup.tile([8, C], i32, tag="p8")
            nc.gpsimd.iota(p8[:], pattern=[[0, C]], base=0, channel_multiplier=1)
            fc8 = setup.tile([8, C], i32, tag="fc8")
            nc.gpsimd.iota(fc8[:], pattern=[[1, C]], base=0, channel_multiplier=0)
            nc.vector.tensor_scalar(fc8[:], fc8[:], 4, None, Alu.arith_shift_right)
            e82 = setup.tile([8, C], i32, tag="e82")
            nc.vector.tensor_tensor(e82[:], p8[:], fc8[:], Alu.is_equal)
            gm2 = const.tile([8, C], f32, tag="gm2")
            icast(gm2[:], e82[:])

            # --- load + transpose the four projection weights (parallel)
            wT = {}
            with tc.tile_pool(name="psum_setup", bufs=1, space="PSUM") as psum_su:
                raws = {}
                for name in ("wq", "wk", "wv", "wo"):
                    raw = setup.tile([C, C], f32r, tag=f"raw_{name}")
                    nc.sync.dma_start(raw[:], w_io[name][:])
                    raws[name] = raw
                for name in ("wq", "wk", "wv", "wo"):
                    pst = psum_su.tile([C, C], f32r, tag=f"pst_{name}")
                    nc.tensor.transpose(pst[:], raws[name][:], ident[:])
                    t = const.tile([C, C], f32r, tag=f"wT_{name}")
                    nc.scalar.copy(t[:], pst[:])
                    wT[name] = t

        # ---------------- main pipeline ---------------------------------
        per_rep_pools = dict(
            xres=ctx.enter_context(tc.tile_pool(name="xres", bufs=2)),
            ypool=ctx.enter_context(tc.tile_pool(name="ypool", bufs=2)),
            sb=ctx.enter_context(tc.tile_pool(name="sb", bufs=2)),
            big=ctx.enter_context(tc.tile_pool(name="bigsb", bufs=2)),
            opool=ctx.enter_context(tc.tile_pool(name="opool", bufs=1)),
            stats=ctx.enter_context(tc.tile_pool(name="stats", bufs=1)),
            ps_kb=ctx.enter_context(tc.tile_pool(name="ps_kb", bufs=1, space="PSUM")),
            ps_bb=ctx.enter_context(tc.tile_pool(name="ps_bb", bufs=1, space="PSUM")),
            ps_q=ctx.enter_context(tc.tile_pool(name="ps_q", bufs=1, space="PSUM")),
            ps_v=ctx.enter_context(tc.tile_pool(name="ps_v", bufs=1, space="PSUM")),
            ps_s=ctx.enter_context(tc.tile_pool(name="ps_s", bufs=1, space="PSUM")),
            ps_o=ctx.enter_context(tc.tile_pool(name="ps_o", bufs=1, space="PSUM")),
        )

        for rep in range(n_reps):
            p = per_rep_pools
            out_acc = p["opool"].tile([C, B * NVOX], f32, tag="out_acc")
            sums = p["stats"].tile([C, B * NTILES], f32, tag="sums")
            ssqs = p["stats"].tile([C, B * NTILES], f32, tag="ssqs")
            dump = p["stats"].tile([C, NT], f32, tag="dump")

            tiles = [(b, t) for b in range(B) for t in range(NTILES)]
            xres_b = {}
            ychunk_state = {}
            fstate = {}
            sstate = {}

            def front1(k):
                """DMAs, Q proj+copy, K h0 proj, V proj+copies."""
                b, t = tiles[k]
                if t == 0:
                    xr = p["xres"].tile([C, NVOX], f32r, tag="xres")
                    nc.sync.dma_start(xr[:], x_io[b])
                    xres_b[b] = xr
                if t % 4 == 0:
                    yc = p["ypool"].tile([C, COND * 4 * NT], f32r, tag="ychunk")
                    ysrc = y_io[b].rearrange("j c v -> c j v")
                    for jh in range(2):
                        nc.sync.dma_start(
                            yc[:, jh * 2 * 4 * NT: (jh + 1) * 2 * 4 * NT]
                            .rearrange("p (j v) -> p j v", j=2),
                            ysrc[:, bass.ts(jh, 2), bass.ts(t // 4, 4 * NT)],
                        )
                    ychunk_state[b] = yc
                ychunk = ychunk_state[b]
                yj_of = lambda j: ychunk[:, j * 4 * NT + (t % 4) * NT:
                                         j * 4 * NT + (t % 4 + 1) * NT]
                xt = xres_b[b][:, ts(t, NT)]
                psQ = p["ps_q"].tile([C, NT], f32, tag="psq")
                mm(psQ[:], wT["wq"][:], xt)
                qsb = p["sb"].tile([C, NT], f32, tag="qsb")
                nc.scalar.activation(qsb[:], psQ[:], Act.Identity,
                                     bias=vecs["bq"][:])
                psKB0 = p["ps_kb"].tile([C, 2 * NT], f32, tag="kb")
                for j in (0, 1):
                    mm(psKB0[:, ts(j, NT)], wT["wk"][:], yj_of(j))
                vbig = p["big"].tile([C, COND * NT], f32, tag="vbig")
                for j in range(COND):
                    psV = p["ps_v"].tile([C, NT], f32, tag="psv")
                    mm(psV[:], wT["wv"][:], yj_of(j))
                    nc.scalar.activation(vbig[:, ts(j, NT)], psV[:],
                                         Act.Identity, bias=vecs["bv"][:])
                qkbig = p["big"].tile([C, COND * NT], f32r, tag="qkbig")
                psS = p["ps_s"].tile([32, NT], f32, tag="pss")
                fstate[k] = (psS, vbig, xt, qsb, qkbig, yj_of, psKB0)

            def front_qk(k, h):
                """QK mul half h + its score matmuls (+ K h1 projections)."""
                psS, vbig, xt, qsb, qkbig, yj_of, psKB = fstate[k]
                for j in (2 * h, 2 * h + 1):
                    nc.vector.scalar_tensor_tensor(
                        qkbig[:, ts(j, NT)], psKB[:, ts(j - 2 * h, NT)],
                        vecs["bk"][:], qsb[:], Alu.add, Alu.mult)
                    mm(psS[:], mask32[:, ts(j, 32)], qkbig[:, ts(j, NT)],
                       start=(j == 0), stop=(j == COND - 1))
                if h == 0:
                    psKB1 = p["ps_kb"].tile([C, 2 * NT], f32, tag="kb")
                    for j in (2, 3):
                        mm(psKB1[:, ts(j - 2, NT)], wT["wk"][:], yj_of(j))
                    fstate[k] = (psS, vbig, xt, qsb, qkbig, yj_of, psKB1)

            def soft(k):
                """exp, Z-matmul, reciprocal, E~ = E * (1/Z)."""
                b, t = tiles[k]
                psS, vbig, xt, qsb, qkbig, yj_of, _ = fstate.pop(k)
                esb = p["sb"].tile([32, NT], f32r, tag="esb")
                nc.scalar.activation(esb[:], psS[:], Act.Exp, scale=0.25)
                psZ = p["ps_q"].tile([32, NT], f32, tag="psq")
                mm(psZ[:], lhsT32[:], esb[:])
                rsb = p["sb"].tile([32, NT], f32, tag="rsb")
                nc.vector.reciprocal(rsb[:], psZ[:])
                etsb = p["sb"].tile([32, NT], f32r, tag="etsb")
                nc.vector.tensor_tensor(etsb[:], esb[:].bitcast(f32),
                                        rsb[:], Alu.mult)
                sstate[k] = (etsb, vbig, xt)

            def back_b(k, h):
                """Broadcast matmuls for half h."""
                etsb, vbig, xt = sstate[k]
                psBB = p["ps_bb"].tile([C, 2 * NT], f32, tag="bb")
                for j in (2 * h, 2 * h + 1):
                    mm(psBB[:, ts(j - 2 * h, NT)], maskb[:, ts(j, C)], etsb[:])
                return psBB

            def back_avm(k, h, psBB, wbig):
                etsb, vbig, xt = sstate[k]
                nc.vector.tensor_tensor(
                    wbig[:, ts(h, 2 * NT)], psBB[:],
                    vbig[:, ts(h, 2 * NT)], Alu.mult)

            def back_out(k, wbig):
                b, t = tiles[k]
                col = b * NTILES + t
                etsb, vbig, xt = sstate.pop(k)
                psO = p["ps_o"].tile([C, NT], f32, tag="pso")
                for j in range(COND):
                    mm(psO[:], wT["wo"][:], wbig[:, ts(j, NT)],
                       start=(j == 0), stop=(j == COND - 1))
                outt = out_acc[:, col * NT: (col + 1) * NT]
                nc.vector.scalar_tensor_tensor(
                    outt, psO[:], vecs["bo"][:], xt.bitcast(f32),
                    Alu.add, Alu.add,
                    accum_out=sums[:, col: col + 1])
                nc.scalar.activation(
                    dump[:], outt, Act.Square,
                    accum_out=ssqs[:, col: col + 1])

            cc_state = {}

            def gn_pre(b):
                """Reduce per-channel stats and launch the AllReduce."""
                ccsb = p["stats"].tile([C, 2], f32, tag=f"ccsb{b}")
                nc.vector.reduce_sum(ccsb[:, 0:1],
                                     sums[:, b * NTILES:(b + 1) * NTILES],
                                     axis=mybir.AxisListType.X)
                nc.vector.reduce_sum(ccsb[:, 1:2],
                                     ssqs[:, b * NTILES:(b + 1) * NTILES],
                                     axis=mybir.AxisListType.X)
                cc_in = dram.tile([C, 2], f32, tag=f"cc_in{b}")
                cc_out = dram.tile([C, 2], f32, tag=f"cc_out{b}")
                nc.sync.dma_start(cc_in[:], ccsb[:])
                nc.gpsimd.collective_compute(
                    "AllReduce", Alu.add,
                    replica_groups=[list(range(NCORES))],
                    ins=[cc_in.opt()], outs=[cc_out.opt()])
                cc_state[b] = cc_out

            def gn_post(b):
                """Stats -> per-channel affine -> rescale out_acc -> store."""
                cc_out = cc_state.pop(b)
                gsb = p["stats"].tile([C, 2], f32, tag=f"gsb{b}")
                nc.sync.dma_start(gsb[:], cc_out[:])
                psG = p["ps_q"].tile([8, 2], f32, tag="psq")
                nc.tensor.matmul(psG[:], lhsT=gmask[:], rhs=gsb[:],
                                 start=True, stop=True)
                msb = p["stats"].tile([8, 2], f32, tag=f"msb{b}")
                nc.vector.tensor_scalar(msb[:], psG[:], 1.0 / N_GROUP, None,
                                        Alu.mult)
                vtmp = p["stats"].tile([8, 2], f32, tag=f"vtmp{b}")
                eps_t = p["stats"].tile([8, 1], f32, tag=f"eps{b}")
                nc.vector.memset(eps_t[:], EPS)
                nc.vector.tensor_tensor(vtmp[:, 0:1], msb[:, 0:1],
                                        msb[:, 0:1], Alu.mult)
                nc.vector.tensor_tensor(vtmp[:, 1:2], msb[:, 1:2],
                                        vtmp[:, 0:1], Alu.subtract)
                nc.scalar.activation(vtmp[:, 0:1], vtmp[:, 1:2], Act.Sqrt,
                                     bias=eps_t[:])
                pstat = p["stats"].tile([8, 2], f32, tag=f"pstat{b}")
                nc.vector.tensor_copy(pstat[:, 0:1], msb[:, 0:1])
                nc.vector.reciprocal(pstat[:, 1:2], vtmp[:, 0:1])
                psP = p["ps_q"].tile([C, 2], f32, tag="psq")
                nc.tensor.matmul(psP[:], lhsT=gm2[:], rhs=pstat[:],
                                 start=True, stop=True)
                scale_b = p["stats"].tile([C, 1], f32, tag=f"scale{b}")
                nc.vector.tensor_tensor(scale_b[:], psP[:, 1:2],
                                        vecs["gamma"][:], Alu.mult)
                negb_b = p["stats"].tile([C, 1], f32, tag=f"negb{b}")
                nc.vector.scalar_tensor_tensor(
                    negb_b[:], psP[:, 0:1], scale_b[:],
                    vecs["beta"][:], Alu.mult, Alu.subtract)
                fin = p["xres"].tile([C, NVOX], f32, tag="xres")
                for t in range(NTILES):
                    src = out_acc[:, (b * NTILES + t) * NT:
                                  (b * NTILES + t + 1) * NT]
                    if t % 2 == 0:
                        nc.vector.tensor_scalar(
                            fin[:, ts(t, NT)], src,
                            scale_b[:], negb_b[:], Alu.mult, Alu.subtract)
                    else:
                        nc.gpsimd.tensor_scalar(
                            fin[:, ts(t, NT)], src,
                            scale_b[:], negb_b[:], Alu.mult, Alu.subtract)
                    if t % 2 == 1:
                        q4 = NVOX // 4
                        qi = t // 2
                        nc.sync.dma_start(
                            out_io[b][:, qi * q4: (qi + 1) * q4],
                            fin[:, qi * q4: (qi + 1) * q4])

            NK = len(tiles)
            for k in range(NK + 2):
                if 1 <= k <= NK:
                    soft(k - 1)
                if k >= 2:
                    psBB0 = back_b(k - 2, 0)
                    wbig = p["big"].tile([C, COND * NT], f32r, tag="qkbig")
                    back_avm(k - 2, 0, psBB0, wbig)
                if k < NK:
                    front1(k)
                    front_qk(k, 0)
                if k >= 2:
                    psBB1 = back_b(k - 2, 1)
                    back_avm(k - 2, 1, psBB1, wbig)
                if k < NK:
                    front_qk(k, 1)
                if k >= 2:
                    back_out(k - 2, wbig)
                    bdone, tdone = tiles[k - 2]
                    if tdone == NTILES - 1:
                        gn_pre(bdone)
                if k >= 5:
                    bpost, tpost = tiles[k - 5]
                    if tpost == NTILES - 1:
                        gn_post(bpost)
            gn_post(B - 1)


    _split_waits(nc)
    return nc


def _shard_inputs(inputs):
    x = np.ascontiguousarray(np.asarray(inputs["decoder_features"], np.float32))
    y = np.ascontiguousarray(
        np.asarray(inputs["skip_connection_features"], np.float32))
    base = {
        "wq": np.ascontiguousarray(np.asarray(inputs["w_q"], np.float32)),
        "wk": np.ascontiguousarray(np.asarray(inputs["w_k"], np.float32)),
        "wv": np.ascontiguousarray(np.asarray(inputs["w_v"], np.float32)),
        "wo": np.ascontiguousarray(np.asarray(inputs["w_o"], np.float32)),
        "bq": np.asarray(inputs["b_q"], np.float32).reshape(C, 1).copy(),
        "bk": np.asarray(inputs["b_k"], np.float32).reshape(C, 1).copy(),
        "bv": np.asarray(inputs["b_v"], np.float32).reshape(C, 1).copy(),
        "bo": np.asarray(inputs["b_o"], np.float32).reshape(C, 1).copy(),
        "gamma": np.asarray(inputs["gn_gamma"], np.float32).reshape(C, 1).copy(),
        "beta": np.asarray(inputs["gn_beta"], np.float32).reshape(C, 1).copy(),
    }
    in_maps = []
    for ci in range(NCORES):
        sl = slice(HS * ci, HS * (ci + 1))
        im = dict(base)
        im["x"] = np.ascontiguousarray(x[:, :, sl]).reshape(B, C, NVOX)
        im["y"] = np.ascontiguousarray(y[:, :, :, sl]).reshape(B, COND, C, NVOX)
        in_maps.append(im)
    return in_maps


class _Runner:
    """Persistent PJRT runner: trace/compile once, execute many times.

    Mirrors concourse.bass2jax.run_bass_via_pjrt's multi-core branch but
    keeps the jitted shard_map callable alive so repeat calls skip
    re-tracing and NEFF recompilation.
    """

    def __init__(self, nc, donate=True):
        import jax
        from jax.sharding import Mesh, PartitionSpec
        from jax.experimental.shard_map import shard_map
        from concourse import bass2jax, mybir

        bass2jax.install_neuronx_cc_hook()
        assert nc.dbg_addr is None
        partition_name = (nc.partition_id_tensor.name
                          if nc.partition_id_tensor else None)
        in_names, out_names, out_avals, zero_outs = [], [], [], []
        for alloc in nc.m.functions[0].allocations:
            if not isinstance(alloc, mybir.MemoryLocationSet):
                continue
            name = alloc.memorylocations[0].name
            if alloc.kind == "ExternalInput":
                if name != partition_name:
                    in_names.append(name)
            elif alloc.kind == "ExternalOutput":
                out_names.append(name)
                shape = tuple(alloc.tensor_shape)
                dtype = mybir.dt.np(alloc.dtype)
                out_avals.append(jax.core.ShapedArray(shape, dtype))
                zero_outs.append(np.zeros(shape, dtype))
        n_params = len(in_names)
        n_outs = len(out_avals)
        in_names.extend(out_names)
        if partition_name is not None:
            in_names.append(partition_name)
        donate_idx = tuple(range(n_params, n_params + n_outs)) if donate else ()

        def _body(*args):
            operands = list(args)
            if partition_name is not None:
                operands.append(bass2jax.partition_id_tensor())
            outs = bass2jax._bass_exec_p.bind(
                *operands,
                out_avals=tuple(out_avals),
                in_names=tuple(in_names),
                out_names=tuple(out_names),
                lowering_input_output_aliases=(),
                sim_require_finite=True,
                sim_require_nnan=True,
                nc=nc,
            )
            return tuple(outs)

        devices = jax.devices()[:NCORES]
        mesh = Mesh(np.asarray(devices), ("core",))
        in_specs = (PartitionSpec("core"),) * (n_params + n_outs)
        out_specs = (PartitionSpec("core"),) * n_outs
        self._fn = jax.jit(
            shard_map(_body, mesh=mesh, in_specs=in_specs,
                      out_specs=out_specs, check_rep=False),
            donate_argnums=donate_idx, keep_unused=True)
        self._in_names = in_names[:n_params]
        self._out_names = out_names
        self._out_avals = out_avals
        self._zero_outs = zero_outs
        self._jax = jax

    def __call__(self, in_maps):
        concat_in = [
            np.concatenate([np.asarray(m[name]) for m in in_maps], axis=0)
            for name in self._in_names
        ]
        concat_zeros = [
            np.zeros((NCORES * z.shape[0], *z.shape[1:]), z.dtype)
            for z in self._zero_outs
        ]
        out_arrs = self._fn(*concat_in, *concat_zeros)
        out_arrs = self._jax.block_until_ready(out_arrs)
        return [
            {
                name: np.asarray(out_arrs[i]).reshape(
                    NCORES, *self._out_avals[i].shape)[c]
                for i, name in enumerate(self._out_names)
            }
            for c in range(NCORES)
        ]


class _Results:
    def __init__(self, results):
        self.results = results


def _get_runner(n_reps=1, donate=True):
    key = (n_reps, donate)
    if key not in _CACHE:
        _CACHE[key] = _Runner(_build(n_reps), donate=donate)
    return _CACHE[key]


def _run(in_maps, n_reps=1):
    return _Results(_get_runner(n_reps)(in_maps))


def kernel(**inputs) -> np.ndarray:
    res = _run(_shard_inputs(inputs))
    out = np.empty((B, C, H, W, D), np.float32)
    for ci in range(NCORES):
        sl = slice(HS * ci, HS * (ci + 1))
        out[:, :, sl] = res.results[ci]["out"].reshape(B, C, HS, W, D)
    return out
